# revision 9
# baseline (speedup 1.0000x reference)
"""Trainium2 Bass kernel for nn_CartesianMoEGenreClassifier.

Strategy: 8-way data parallel over the batch (2 sequences = 1024 tokens per
core), all weights replicated.  Activations live feature-major in SBUF as
[128, 6, 1024] (d-partition, d-subtile, token).  MoE layers computed densely
(all 4 experts) with per-token masked accumulation into the W2 PSUM group,
exactly matching the reference's dense-then-select semantics.  Matmuls use
float32r (full-rate, ~12-bit mantissa).  The tiny aux-loss reduction is done
on host from the gating logits the device dumps (8 KB/core), as part of
un-sharding.
"""
import sys, os, types

sys.path.insert(0, "/opt/trn_rl_repo")

import numpy as np

try:  # NTFF profile hook shim (missing antenv.axon_hooks in this image)
    import antenv  # noqa: F401
    if "antenv.axon_hooks" not in sys.modules:
        _hm = types.ModuleType("antenv.axon_hooks")
        _h = [None]
        _hm.set_axon_ntff_profile_hook = lambda hook: _h.__setitem__(0, hook)
        _hm.get_axon_ntff_profile_hook = lambda: _h[0]
        sys.modules["antenv.axon_hooks"] = _hm
        try:
            from trn_agent_boot.trn_boot import _ntff_profile_via_ctypes
            _hm.set_axon_ntff_profile_hook(
                _ntff_profile_via_ctypes("/opt/axon/libaxon_pjrt.so"))
        except Exception:
            pass
except Exception:
    pass

import concourse.bass as bass
import concourse.tile as tile
from concourse import bacc, mybir
from concourse.bass_utils import run_bass_kernel_spmd
from concourse.masks import make_identity

P = 128
V, D, H, L, FF, NA, NB, S, C, BS = 32000, 768, 12, 6, 3072, 2, 2, 512, 10, 16
MOE = (1, 3, 5)
NCORES = 8
T = 1024            # tokens per core
KS = D // P         # 6 contraction subtiles
DH = 64             # head dim
SEQ = 2             # sequences per core
JH = FF // P        # 24 g-subtiles
F32 = mybir.dt.float32
F32R = mybir.dt.float32r
BF16 = mybir.dt.bfloat16
I32 = mybir.dt.int32
AF = mybir.ActivationFunctionType
OP = mybir.AluOpType

NLAYERS = int(os.environ.get("K_LAYERS", "6"))
TAP = os.environ.get("K_TAP", "")  # "", "emb", "xa", "x" — debug tap point


def _lhsT(w):
    """[K, M] -> [128, K//128, M] feature-major lhsT layout."""
    K, M = w.shape
    return np.ascontiguousarray(w.reshape(K // P, P, M).transpose(1, 0, 2))


def _col(v):
    """[n*128] bias vector -> [128, n]."""
    return np.ascontiguousarray(v.reshape(-1, P).T)


def build(nlayers=NLAYERS):
    nc = bacc.Bacc(None, target_bir_lowering=False)
    dt = {}

    def inp(name, shape, dtype=F32R):
        dt[name] = nc.dram_tensor(name, list(shape), dtype, kind="ExternalInput")
        return dt[name]

    def outp(name, shape, dtype=F32):
        dt[name] = nc.dram_tensor(name, list(shape), dtype, kind="ExternalOutput")
        return dt[name]

    inp("ids", [P, 8], I32)
    inp("tok_emb", [V, D])
    inp("pos_fm", [P, KS, S])
    for l in range(nlayers):
        inp(f"wqkT_{l}", [12, P, KS, P])
        inp(f"wvT_{l}", [2, P, KS, 384])
        inp(f"woT_{l}", [6, P, KS, P])
        inp(f"bqk_{l}", [P, 12], F32)
        inp(f"bo_{l}", [P, 6], F32)
        for nm in ("ln1g", "ln1b", "ln2g", "ln2b"):
            inp(f"{nm}_{l}", [P, KS], F32)
        if l in MOE:
            inp(f"mW1T_{l}", [4, 2 * JH, P, KS, P])
            inp(f"mB1_{l}", [P, 4, 2 * JH], F32)
            inp(f"mW2T_{l}", [4, JH, P, D])
            inp(f"mB2r_{l}", [4, 1, D])
            inp(f"gwT_{l}", [P, KS, P])
            inp(f"gb_{l}", [1, 4], F32)
        else:
            inp(f"dW1T_{l}", [2 * JH, P, KS, P])
            inp(f"dB1_{l}", [P, 2 * JH], F32)
            inp(f"dW2T_{l}", [JH, P, D])
            inp(f"dB2_{l}", [P, 6], F32)
    inp("cW1T", [6, P, KS, P])
    inp("cB1", [P, 6], F32)
    inp("cW2T", [P, KS, C])
    inp("cB2", [C, 1], F32)

    outp("logits_o", [C, SEQ])
    outp("zout", [3, 4, T])
    if TAP:
        outp("xtap", [P, KS, T])

    with nc.allow_low_precision(reason="f32r activations by design; masks exact"), \
         tile.TileContext(nc) as tc:
        with tc.tile_pool(name="sb", bufs=1) as sb, \
             tc.tile_pool(name="psA", bufs=6, space="PSUM") as psA, \
             tc.tile_pool(name="psB", bufs=2, space="PSUM") as psB:

            # ---- constants ----
            ident_f = sb.tile([P, P], F32, tag="identf", bufs=1)
            make_identity(nc, ident_f[:])
            ident_r = sb.tile([P, P], F32R, tag="identr", bufs=1)
            nc.scalar.activation(ident_r[:], ident_f[:], AF.Identity)
            ones_f = sb.tile([P, P], F32, tag="onesf", bufs=1)
            nc.vector.memset(ones_f[:], 1.0)
            ones_r = sb.tile([P, P], F32R, tag="onesr", bufs=1)
            nc.scalar.activation(ones_r[:], ones_f[:], AF.Identity)
            # ones_r[:, 0:1] is a [128,1] lhsT column; ones_r[0:1, :] a [1,128] row

            ids_sb = sb.tile([P, 8], I32, tag="ids", bufs=1)
            nc.sync.dma_start(ids_sb[:], dt["ids"][:])

            # ---- x resident tile ----
            xt = sb.tile([P, KS, T], F32R, tag="x", bufs=1)

            # ---- embedding: gather + transpose + pos add ----
            pos_sb = sb.tile([P, KS, S], F32R, tag="sq", bufs=1)
            nc.sync.dma_start(pos_sb[:], dt["pos_fm"][:])
            for ti in range(8):
                xg = sb.tile([P, D], F32R, tag="sil", bufs=2)
                nc.gpsimd.indirect_dma_start(
                    out=xg[:], out_offset=None, in_=dt["tok_emb"][:],
                    in_offset=bass.IndirectOffsetOnAxis(
                        ap=ids_sb[:, ti:ti + 1], axis=0))
                for sub in range(KS):
                    pst = psB.tile([P, P], F32R, tag="psB")
                    nc.tensor.transpose(pst[:], xg[:, sub * P:(sub + 1) * P],
                                        ident_r[:])
                    nc.scalar.activation(xt[:, sub, ti * P:(ti + 1) * P], pst[:],
                                         AF.Identity)
            for s in range(SEQ):
                nc.vector.tensor_tensor(xt[:, :, s * S:(s + 1) * S],
                                        xt[:, :, s * S:(s + 1) * S],
                                        pos_sb[:, :, :], op=OP.add)
            if TAP == "emb":
                nc.gpsimd.dma_start(dt["xtap"][:], xt[:].bitcast(F32))

            # ---- helpers ----
            def layernorm(g_sb, b_sb):
                """In-place LN over d (partitions+subtiles) of xt."""
                s1 = sb.tile([1, T], F32, tag="v_s1", bufs=1)
                s2 = sb.tile([1, T], F32, tag="v_s2", bufs=1)
                sd = sb.tile([1, T], F32, tag="v_sd", bufs=1)
                inv = sb.tile([1, T], F32R, tag="v_inv", bufs=1)
                minv = sb.tile([1, T], F32R, tag="v_minv", bufs=1)
                for t in range(2):
                    w = slice(t * 512, (t + 1) * 512)
                    sq = sb.tile([P, KS, 512], F32R, tag="sq", bufs=1)
                    nc.scalar.activation(sq[:], xt[:, :, w], AF.Square)
                    ps1 = psB.tile([1, 512], F32, tag="psB")
                    for k in range(KS):
                        nc.tensor.matmul(ps1[:], ones_r[:, 0:1], xt[:, k, w],
                                         start=(k == 0), stop=(k == KS - 1))
                    ps2 = psB.tile([1, 512], F32, tag="psB")
                    for k in range(KS):
                        nc.tensor.matmul(ps2[:], ones_r[:, 0:1], sq[:, k, :],
                                         start=(k == 0), stop=(k == KS - 1))
                    nc.scalar.activation(s1[0:1, w], ps1[:], AF.Identity,
                                         scale=1.0 / D)
                    nc.scalar.activation(s2[0:1, w], ps2[:], AF.Identity,
                                         scale=1.0 / D)
                # var = s2 - s1^2 ; sd = sqrt(var + eps); inv = 1/sd; minv = s1*inv
                nc.vector.tensor_tensor(sd[:], s1[:], s1[:], op=OP.mult)
                nc.vector.tensor_tensor(sd[:], s2[:], sd[:], op=OP.subtract)
                nc.vector.tensor_scalar(sd[:], sd[:], 1e-5, None, op0=OP.add)
                nc.scalar.activation(sd[:], sd[:], AF.Sqrt)
                nc.vector.reciprocal(inv[:], sd[:])
                nc.vector.tensor_tensor(minv[:], s1[:], inv[:], op=OP.mult)
                for t in range(2):
                    w = slice(t * 512, (t + 1) * 512)
                    psi = psA.tile([P, 512], F32, tag="psA")
                    nc.tensor.matmul(psi[:], ones_r[0:1, :], inv[0:1, w],
                                     start=True, stop=True)
                    psm = psA.tile([P, 512], F32, tag="psA")
                    nc.tensor.matmul(psm[:], ones_r[0:1, :], minv[0:1, w],
                                     start=True, stop=True)
                    bi = psi[:, None, :].to_broadcast([P, KS, 512])
                    bm = psm[:, None, :].to_broadcast([P, KS, 512])
                    nc.vector.tensor_tensor(xt[:, :, w], xt[:, :, w], bi,
                                            op=OP.mult)
                    nc.vector.tensor_tensor(xt[:, :, w], xt[:, :, w], bm,
                                            op=OP.subtract)
                for sub in range(KS):
                    nc.vector.tensor_scalar(
                        xt[:, sub, :], xt[:, sub, :],
                        g_sb[:, sub:sub + 1], b_sb[:, sub:sub + 1],
                        op0=OP.mult, op1=OP.add)

            def attention(l):
                bqk = sb.tile([P, 12], F32, tag="b_bqk", bufs=1)
                nc.sync.dma_start(bqk[:], dt[f"bqk_{l}"][:])
                bo = sb.tile([P, 6], F32, tag="b_bo", bufs=1)
                nc.sync.dma_start(bo[:], dt[f"bo_{l}"][:])
                ofm = sb.tile([P, KS, T], F32R, tag="ofm", bufs=1)
                for s in range(SEQ):
                    w = slice(s * 512, (s + 1) * 512)
                    # Q,K feature-major for this seq
                    qk = sb.tile([P, 12, 512], F32R, tag="qk", bufs=1)
                    for dtile in range(12):
                        wt = sb.tile([P, KS, P], F32R, tag="wA", bufs=2)
                        nc.sync.dma_start(wt[:], dt[f"wqkT_{l}"][dtile])
                        ps = psA.tile([P, 512], F32, tag="psA")
                        for k in range(KS):
                            nc.tensor.matmul(ps[:], wt[:, k, :], xt[:, k, w],
                                             start=(k == 0), stop=(k == KS - 1))
                        nc.scalar.activation(qk[:, dtile, :], ps[:], AF.Identity,
                                             bias=bqk[:, dtile:dtile + 1])
                    # V token-major, padded per head with ones col at 64
                    vpad = sb.tile([P, 4, H, DH + 1], F32R, tag="vpad", bufs=1)
                    nc.scalar.activation(vpad[:, :, :, DH], ones_f[:, 0:48],
                                         AF.Identity)
                    for ti in range(4):
                        for nh in range(2):
                            wv = sb.tile([P, KS, 384], F32R, tag="wV", bufs=1)
                            nc.sync.dma_start(wv[:], dt[f"wvT_{l}"][nh])
                            psv = psB.tile([P, 384], F32, tag="psB")
                            tok = s * 512 + ti * P
                            for k in range(KS):
                                nc.tensor.matmul(
                                    psv[:], xt[:, k, tok:tok + P], wv[:, k, :],
                                    start=(k == 0), stop=(k == KS - 1))
                            nc.vector.tensor_copy(
                                vpad[:, ti, nh * 6:(nh + 1) * 6, 0:DH],
                                psv[:].rearrange("p (h d) -> p h d", h=6))
                    # scores -> exp -> attnV per head
                    for h in range(H):
                        hb = (h % 2) * DH
                        qsub, ksub = h // 2, 6 + h // 2
                        po = psA.tile([DH + 1, 512], F32, tag="psA")
                        for kt in range(4):
                            pss = psA.tile([P, 512], F32, tag="psA")
                            nc.tensor.matmul(
                                pss[:],
                                qk[hb:hb + DH, ksub, kt * P:(kt + 1) * P],
                                qk[hb:hb + DH, qsub, :],
                                start=True, stop=True)
                            ex = sb.tile([P, 512], F32R, tag="expT", bufs=3)
                            nc.scalar.activation(ex[:], pss[:], AF.Exp,
                                                 scale=0.125)
                            nc.tensor.matmul(po[:], vpad[:, kt, h, :], ex[:],
                                             start=(kt == 0), stop=(kt == 3))
                        rcp = sb.tile([1, 512], F32, tag="rcp", bufs=2)
                        nc.vector.reciprocal(rcp[:], po[DH:DH + 1, :])
                        rcpb = sb.tile([DH, 512], F32, tag="rcpb", bufs=2)
                        nc.gpsimd.partition_broadcast(rcpb[:], rcp[0:1, :])
                        nc.vector.tensor_tensor(
                            ofm[hb:hb + DH, h // 2, w], po[0:DH, :], rcpb[:],
                            op=OP.mult)
                # wo + residual (in-place into xt)
                for dtile in range(6):
                    wt = sb.tile([P, KS, P], F32R, tag="wA", bufs=2)
                    nc.sync.dma_start(wt[:], dt[f"woT_{l}"][dtile])
                    for t in range(2):
                        w = slice(t * 512, (t + 1) * 512)
                        ps = psA.tile([P, 512], F32, tag="psA")
                        for k in range(KS):
                            nc.tensor.matmul(ps[:], wt[:, k, :], ofm[:, k, w],
                                             start=(k == 0), stop=(k == KS - 1))
                        nc.vector.scalar_tensor_tensor(
                            xt[:, dtile, w], ps[:], bo[:, dtile:dtile + 1],
                            xt[:, dtile, w], op0=OP.add, op1=OP.add)

            def ffn_block(l, w1name, w2name, b1_ap, b2r_name, nexp, gwname):
                """Dense FFN (nexp=1) or masked dense MoE (nexp=4).

                b1_ap(e) -> [P, 2*JH] bias AP; for MoE b2 is applied via
                K=1 matmuls of mB2r x mask-row; dense applies dB2 in the
                residual step."""
                mi = MOE.index(l) if nexp == 4 else None
                if nexp == 4:
                    gw = sb.tile([P, KS, P], F32R, tag="wA", bufs=2)
                    nc.sync.dma_start(gw[:], dt[gwname][:])
                    gb = sb.tile([1, 4], F32, tag="b_gb", bufs=1)
                    nc.sync.dma_start(gb[:], dt[f"gb_{l}"][:])
                    zr = sb.tile([1, 4 * T], F32, tag="v_zr", bufs=1)
                    for t in range(2):
                        w = slice(t * 512, (t + 1) * 512)
                        psg = psA.tile([P, 512], F32, tag="psA")
                        for k in range(KS):
                            nc.tensor.matmul(psg[:], gw[:, k, :], xt[:, k, w],
                                             start=(k == 0), stop=(k == KS - 1))
                        for j in range(4):
                            nc.scalar.activation(
                                zr[0:1, j * T + t * 512: j * T + (t + 1) * 512],
                                psg[32 * j:32 * j + 1, :], AF.Identity,
                                bias=gb[0:1, j:j + 1])
                    for j in range(4):
                        nc.sync.dma_start(dt["zout"][mi, j, :],
                                          zr[0:1, j * T:(j + 1) * T])
                    mrow = sb.tile([1, 2 * T], BF16, tag="v_mrow", bufs=1)
                    nc.vector.tensor_tensor(mrow[0:1, 0:T], zr[0:1, T:2 * T],
                                            zr[0:1, 0:T], op=OP.is_gt)
                    nc.vector.tensor_tensor(mrow[0:1, T:2 * T],
                                            zr[0:1, 3 * T:4 * T],
                                            zr[0:1, 2 * T:3 * T], op=OP.is_gt)
                    mab = sb.tile([P, T], BF16, tag="mab", bufs=1)
                    nc.gpsimd.partition_broadcast(mab[:], mrow[0:1, 0:T])
                    mbb = sb.tile([P, T], BF16, tag="mbb", bufs=1)
                    nc.gpsimd.partition_broadcast(mbb[:], mrow[0:1, T:2 * T])
                    b2r = sb.tile([1, D], F32R, tag="b2r", bufs=2)

                for t in range(2):
                    w = slice(t * 512, (t + 1) * 512)
                    yps = []
                    for _yi in range(6):
                        ypt = psA.tile([P, 512], F32, tag="psA", name=f"yps{_yi}")
                        yps.append(ypt)
                    if nexp == 4:
                        # m3 = mA*mB chunks (expert 3 mask), then derive others
                        m3ch = sb.tile([P, 512], BF16, tag="mtmp", bufs=2)
                        nc.vector.tensor_tensor(m3ch[:], mab[:, w], mbb[:, w],
                                                op=OP.mult)
                        m3r = sb.tile([1, 512], F32R, tag="v_mtmp", bufs=2)
                        nc.vector.tensor_tensor(
                            m3r[:], mrow[0:1, t * 512:(t + 1) * 512],
                            mrow[0:1, T + t * 512:T + (t + 1) * 512], op=OP.mult)
                    for e in range(nexp):
                        if nexp == 4:
                            # per-(e,t) mask chunk [P,512] and mask row [1,512]
                            mch = sb.tile([P, 512], BF16, tag="mch", bufs=2)
                            mer = sb.tile([1, 512], F32R, tag="v_mer", bufs=2)
                            ma, mb = mab[:, w], mbb[:, w]
                            mar = mrow[0:1, t * 512:(t + 1) * 512]
                            mbr = mrow[0:1, T + t * 512:T + (t + 1) * 512]
                            if e == 0:
                                # 1 - mA - mB + m3
                                nc.vector.tensor_tensor(mch[:], ma, mb,
                                                        op=OP.add)
                                nc.vector.tensor_tensor(mch[:], mch[:], m3ch[:],
                                                        op=OP.subtract)
                                nc.vector.tensor_scalar(mch[:], mch[:],
                                                        -1.0, 1.0,
                                                        op0=OP.mult, op1=OP.add)
                                nc.vector.tensor_tensor(mer[:], mar, mbr,
                                                        op=OP.add)
                                nc.vector.tensor_tensor(mer[:], mer[:], m3r[:],
                                                        op=OP.subtract)
                                nc.vector.tensor_scalar(mer[:], mer[:],
                                                        -1.0, 1.0,
                                                        op0=OP.mult, op1=OP.add)
                            elif e == 1:
                                nc.vector.tensor_tensor(mch[:], mb, m3ch[:],
                                                        op=OP.subtract)
                                nc.vector.tensor_tensor(mer[:], mbr, m3r[:],
                                                        op=OP.subtract)
                            elif e == 2:
                                nc.vector.tensor_tensor(mch[:], ma, m3ch[:],
                                                        op=OP.subtract)
                                nc.vector.tensor_tensor(mer[:], mar, m3r[:],
                                                        op=OP.subtract)
                            else:
                                mch = m3ch
                                mer = m3r
                        b1 = b1_ap(e)
                        for j in range(JH):
                            wb = sb.tile([P, KS, P], F32R, tag="wA", bufs=2)
                            src = dt[w1name]
                            nc.sync.dma_start(
                                wb[:], src[e, JH + j] if nexp == 4 else src[JH + j])
                            psb_ = psB.tile([P, 512], F32, tag="psB")
                            for k in range(KS):
                                nc.tensor.matmul(psb_[:], wb[:, k, :], xt[:, k, w],
                                                 start=(k == 0),
                                                 stop=(k == KS - 1))
                            sil = sb.tile([P, 512], F32R, tag="sil", bufs=2)
                            nc.scalar.activation(sil[:], psb_[:], AF.Silu,
                                                 bias=b1[:, JH + j:JH + j + 1])
                            wa = sb.tile([P, KS, P], F32R, tag="wA", bufs=2)
                            nc.sync.dma_start(
                                wa[:], src[e, j] if nexp == 4 else src[j])
                            psa_ = psB.tile([P, 512], F32, tag="psB")
                            for k in range(KS):
                                nc.tensor.matmul(psa_[:], wa[:, k, :], xt[:, k, w],
                                                 start=(k == 0),
                                                 stop=(k == KS - 1))
                            gj = sb.tile([P, 512], F32R, tag="gj", bufs=2)
                            nc.vector.scalar_tensor_tensor(
                                gj[:], psa_[:], b1[:, j:j + 1], sil[:],
                                op0=OP.add, op1=OP.mult)
                            if nexp == 4:
                                nc.vector.tensor_tensor(gj[:], gj[:], mch[:],
                                                        op=OP.mult)
                            w2 = sb.tile([P, D], F32R, tag="w2", bufs=2)
                            src2 = dt[w2name]
                            nc.sync.dma_start(
                                w2[:], src2[e, j] if nexp == 4 else src2[j])
                            for dtile in range(6):
                                nc.tensor.matmul(
                                    yps[dtile][:], w2[:, dtile * P:(dtile + 1) * P],
                                    gj[:],
                                    start=(e == 0 and j == 0),
                                    stop=(nexp == 1 and j == JH - 1))
                        if nexp == 4:
                            nc.sync.dma_start(b2r[:], dt[b2r_name][e])
                            for dtile in range(6):
                                nc.tensor.matmul(
                                    yps[dtile][:],
                                    b2r[0:1, dtile * P:(dtile + 1) * P],
                                    mer[0:1, :], start=False, stop=(e == 3))
                    # residual add (+ dense b2) in place
                    if nexp == 1:
                        b2 = sb.tile([P, 6], F32, tag="b_b2", bufs=1)
                        nc.sync.dma_start(b2[:], dt[f"dB2_{l}"][:])
                        for dtile in range(6):
                            nc.vector.scalar_tensor_tensor(
                                xt[:, dtile, w], yps[dtile][:],
                                b2[:, dtile:dtile + 1], xt[:, dtile, w],
                                op0=OP.add, op1=OP.add)
                    else:
                        for dtile in range(6):
                            nc.vector.tensor_tensor(
                                xt[:, dtile, w], yps[dtile][:], xt[:, dtile, w],
                                op=OP.add)

            # ---- layers ----
            for l in range(nlayers):
                lng1 = sb.tile([P, KS], F32, tag="b_g1", bufs=1)
                nc.sync.dma_start(lng1[:], dt[f"ln1g_{l}"][:])
                lnb1 = sb.tile([P, KS], F32, tag="b_b1v", bufs=1)
                nc.sync.dma_start(lnb1[:], dt[f"ln1b_{l}"][:])
                lng2 = sb.tile([P, KS], F32, tag="b_g2", bufs=1)
                nc.sync.dma_start(lng2[:], dt[f"ln2g_{l}"][:])
                lnb2 = sb.tile([P, KS], F32, tag="b_b2v", bufs=1)
                nc.sync.dma_start(lnb2[:], dt[f"ln2b_{l}"][:])
                attention(l)
                layernorm(lng1, lnb1)
                if TAP == "xa" and l == nlayers - 1:
                    nc.gpsimd.dma_start(dt["xtap"][:], xt[:].bitcast(F32))
                if l in MOE:
                    mb1 = sb.tile([P, 4, 2 * JH], F32, tag="b_mb1", bufs=1)
                    nc.sync.dma_start(mb1[:], dt[f"mB1_{l}"][:])
                    ffn_block(l, f"mW1T_{l}", f"mW2T_{l}",
                              lambda e, _m=mb1: _m[:, e, :], f"mB2r_{l}", 4,
                              f"gwT_{l}")
                else:
                    db1 = sb.tile([P, 2 * JH], F32, tag="b_db1", bufs=1)
                    nc.sync.dma_start(db1[:], dt[f"dB1_{l}"][:])
                    ffn_block(l, f"dW1T_{l}", f"dW2T_{l}",
                              lambda e, _d=db1: _d, None, 1, None)
                layernorm(lng2, lnb2)
                if TAP == "x" and l == nlayers - 1:
                    nc.gpsimd.dma_start(dt["xtap"][:], xt[:].bitcast(F32))

            # ---- classifier ----
            rep_f = sb.tile([P, KS, SEQ], F32, tag="rep", bufs=1)
            rep_r = sb.tile([P, KS, SEQ], F32R, tag="repr", bufs=1)
            for s in range(SEQ):
                nc.vector.tensor_reduce(
                    rep_f[:, :, s], xt[:, :, s * S:(s + 1) * S],
                    axis=mybir.AxisListType.X, op=OP.add)
            nc.scalar.activation(rep_r[:], rep_f[:], AF.Identity, scale=1.0 / S)
            cb1 = sb.tile([P, 6], F32, tag="b_cb1", bufs=1)
            nc.sync.dma_start(cb1[:], dt["cB1"][:])
            cb2 = sb.tile([C, 1], F32, tag="b_cb2", bufs=1)
            nc.sync.dma_start(cb2[:], dt["cB2"][:])
            cw2 = sb.tile([P, KS, C], F32R, tag="cw2", bufs=1)
            nc.sync.dma_start(cw2[:], dt["cW2T"][:])
            hcls = sb.tile([P, KS, SEQ], F32R, tag="hcls", bufs=1)
            for dtile in range(6):
                wt = sb.tile([P, KS, P], F32R, tag="wA", bufs=2)
                nc.sync.dma_start(wt[:], dt["cW1T"][dtile])
                psc = psB.tile([P, SEQ], F32, tag="psB")
                for k in range(KS):
                    nc.tensor.matmul(psc[:], wt[:, k, :], rep_r[:, k, :],
                                     start=(k == 0), stop=(k == KS - 1))
                nc.scalar.activation(hcls[:, dtile, :], psc[:], AF.Relu,
                                     bias=cb1[:, dtile:dtile + 1])
            psl = psB.tile([C, SEQ], F32, tag="psB")
            for k in range(KS):
                nc.tensor.matmul(psl[:], cw2[:, k, :], hcls[:, k, :],
                                 start=(k == 0), stop=(k == KS - 1))
            lg = sb.tile([C, SEQ], F32, tag="lg", bufs=1)
            nc.scalar.activation(lg[:], psl[:], AF.Identity, bias=cb2[:, 0:1])
            nc.sync.dma_start(dt["logits_o"][:], lg[:])

    nc.compile()
    return nc


_NC_CACHE = {}


def _get_nc():
    key = (NLAYERS, TAP)
    if key not in _NC_CACHE:
        _NC_CACHE[key] = build(NLAYERS)
    return _NC_CACHE[key]


def _prep_shared(I):
    """Host-side weight layout prep (shared across cores)."""
    f32 = np.float32
    sh = {}
    sh["tok_emb"] = np.ascontiguousarray(I["tok_emb"], dtype=f32)
    sh["pos_fm"] = np.ascontiguousarray(
        I["pos_emb"].T.reshape(KS, P, S).transpose(1, 0, 2))
    for l in range(L):
        wqkv = I["wqkv"][l]
        qkT = _lhsT(np.ascontiguousarray(wqkv[:2 * D].T))        # [128,6,1536]
        sh[f"wqkT_{l}"] = np.ascontiguousarray(
            qkT.reshape(P, KS, 12, P).transpose(2, 0, 1, 3))
        vT = _lhsT(np.ascontiguousarray(wqkv[2 * D:].T))         # [128,6,768]
        sh[f"wvT_{l}"] = np.ascontiguousarray(
            vT.reshape(P, KS, 2, 384).transpose(2, 0, 1, 3))
        woT = _lhsT(np.ascontiguousarray(I["wo"][l].T))
        sh[f"woT_{l}"] = np.ascontiguousarray(
            woT.reshape(P, KS, 6, P).transpose(2, 0, 1, 3))
        sh[f"bqk_{l}"] = _col(I["bqkv"][l][:2 * D])
        bo_eff = I["bo"][l] + I["wo"][l] @ I["bqkv"][l][2 * D:]
        sh[f"bo_{l}"] = _col(bo_eff.astype(f32))
        for nm in ("ln1g", "ln1b", "ln2g", "ln2b"):
            sh[f"{nm}_{l}"] = _col(I[nm][l])
        if l in MOE:
            mi = MOE.index(l)
            w1 = I["mW1"][mi]                                    # [4,6144,768]
            sh[f"mW1T_{l}"] = np.ascontiguousarray(
                np.stack([_lhsT(np.ascontiguousarray(w1[e].T)).reshape(
                    P, KS, 2 * JH, P).transpose(2, 0, 1, 3) for e in range(4)]))
            sh[f"mB1_{l}"] = np.ascontiguousarray(
                I["mB1"][mi].reshape(4, 2 * JH, P).transpose(2, 0, 1))
            w2 = I["mW2"][mi]                                    # [4,768,3072]
            sh[f"mW2T_{l}"] = np.ascontiguousarray(
                np.stack([_lhsT(np.ascontiguousarray(w2[e].T)).transpose(1, 0, 2)
                          for e in range(4)]))
            sh[f"mB2r_{l}"] = np.ascontiguousarray(
                I["mB2"][mi].reshape(4, 1, D))
            gw = np.concatenate([I["gAw"][mi], I["gBw"][mi]], 0)  # [4, 768]
            gpad = np.zeros((D, P), f32)
            for j in range(4):
                gpad[:, 32 * j] = gw[j]
            sh[f"gwT_{l}"] = _lhsT(gpad)
            sh[f"gb_{l}"] = np.concatenate(
                [I["gAb"][mi], I["gBb"][mi]]).reshape(1, 4).astype(f32)
        else:
            di = [i for i in range(L) if i not in MOE].index(l)
            w1 = I["dW1"][di]
            sh[f"dW1T_{l}"] = np.ascontiguousarray(
                _lhsT(np.ascontiguousarray(w1.T)).reshape(
                    P, KS, 2 * JH, P).transpose(2, 0, 1, 3))
            sh[f"dB1_{l}"] = _col(I["dB1"][di])
            w2 = I["dW2"][di]
            sh[f"dW2T_{l}"] = np.ascontiguousarray(
                _lhsT(np.ascontiguousarray(w2.T)).transpose(1, 0, 2))
            sh[f"dB2_{l}"] = _col(I["dB2"][di])
    sh["cW1T"] = np.ascontiguousarray(
        _lhsT(np.ascontiguousarray(I["cW1"].T)).reshape(
            P, KS, 6, P).transpose(2, 0, 1, 3))
    sh["cB1"] = _col(I["cB1"])
    sh["cW2T"] = _lhsT(np.ascontiguousarray(I["cW2"].T))
    sh["cB2"] = I["cB2"].reshape(C, 1).astype(f32)
    return sh


def kernel(**inputs):
    I = {k: np.asarray(v) for k, v in inputs.items()}
    sh = _prep_shared(I)
    ids_all = I["input_ids"].astype(np.int32)                    # [16, 512]
    in_maps = []
    for c in range(NCORES):
        m = dict(sh)
        flat = ids_all[2 * c:2 * c + 2].reshape(-1)              # [1024]
        m["ids"] = np.ascontiguousarray(flat.reshape(8, P).T)
        in_maps.append(m)
    nc = _get_nc()
    res = run_bass_kernel_spmd(nc, in_maps, core_ids=list(range(NCORES)))

    logits = np.zeros((BS, C), np.float32)
    zs = []  # per-layer gating logits [4, 8192]
    for mi in range(3):
        zs.append(np.concatenate(
            [res.results[c]["zout"][mi] for c in range(NCORES)], axis=1))
    for c in range(NCORES):
        lo = res.results[c]["logits_o"]                          # [10, 2]
        logits[2 * c] = lo[:, 0]
        logits[2 * c + 1] = lo[:, 1]

    total_aux = np.float64(0.0)
    for mi in range(3):
        z = zs[mi].astype(np.float64)                            # [4, 8192]
        for (z0, z1, n) in ((z[0], z[1], NA), (z[2], z[3], NB)):
            zm = np.maximum(z0, z1)
            e0, e1 = np.exp(z0 - zm), np.exp(z1 - zm)
            p1 = e1 / (e0 + e1)
            p0 = 1.0 - p1
            i1 = (z1 > z0)
            frac1 = i1.mean()
            total_aux += n * (p0.mean() * (1.0 - frac1) + p1.mean() * frac1)
    return logits, np.float32(total_aux)


if __name__ == "__main__":
    d = np.load("/root/problem/ref_inputs.npz")
    out = kernel(**{k: d[k] for k in d.files})
    print(out[0][:2], out[1])
